# revision 3
# baseline (speedup 1.0000x reference)
"""MOLELinear (mixture-of-linear-experts) Trainium2 kernel, v2.

Math (per group g): out_g = x_g @ (sum_e c[g,e] W_e + W_sh).T + (sum_e c[g,e] b_e + b_sh)

Sharding: data-parallel over the 32 groups -> 4 groups (8192 tokens) per core,
expert weights replicated. Host does layout-only prep (transpose/stack/permute,
no arithmetic); all FLOPs run on device.

v2 design (from baseline trace analysis):
  - All GEMM operands in float32r: 1 cyc/row on PE for N>=256, no cast ops.
  - DMA layouts are one contiguous 8KB segment per partition: x is staged
    [kpart, ch, kt, j, p] with token t = 4p + j inside each 512-token chunk,
    which makes the output rows per partition contiguous too.
  - Weights staged [kpart, slot, kt, o] with slot 0 = shared, 1..8 = experts;
    loaded in 3 chunks of 3 slots so DVE mixing starts early.
  - Weight mixing on DVE as 32 STT ops of free-dim 2048 (whole k range per op).
  - Bias path on PE (K=9 mix matmul, then K=1 ones-row matmul adds bias into
    each PSUM accumulation).
  - Engine split: Sync issues x/w/small loads, ScalarE drains PSUM and issues
    out stores, DVE mixes, PE does GEMM.
"""
import numpy as np

import concourse.bacc as bacc
import concourse.mybir as mybir
from concourse.alu_op_type import AluOpType
from concourse.tile import TileContext
from concourse.bass_utils import run_bass_kernel_spmd

N_CORES = 8
IN_F = 512
OUT_F = 512
N_EXPERTS = 8
N_GROUPS = 32
TOK_PER_GROUP = 2048
G_PER_CORE = N_GROUPS // N_CORES          # 4
TOK_PER_CORE = G_PER_CORE * TOK_PER_GROUP  # 8192
KT = IN_F // 128                           # 4 k-tiles
N_CH = TOK_PER_CORE // 512                 # 16 chunks of 512 tokens
NSL = N_EXPERTS + 1                        # weight slots: [shared, e0..e7]
F32 = mybir.dt.float32
F32R = mybir.dt.float32r

_CACHE = {}


def _build():
    nc = bacc.Bacc(trn_type="TRN2")
    xs = nc.dram_tensor("xs", (128, N_CH * 2048), F32R, kind="ExternalInput")
    wt = nc.dram_tensor("wt", (128, NSL * 2048), F32R, kind="ExternalInput")
    cb = nc.dram_tensor("cb", (128, G_PER_CORE * N_EXPERTS), F32R, kind="ExternalInput")
    cx = nc.dram_tensor("cx", (NSL, G_PER_CORE), F32R, kind="ExternalInput")
    ball = nc.dram_tensor("ball", (NSL, OUT_F), F32R, kind="ExternalInput")
    ones = nc.dram_tensor("ones", (1, 128), F32R, kind="ExternalInput")
    out = nc.dram_tensor("out", (TOK_PER_CORE, OUT_F), F32, kind="ExternalOutput")

    with TileContext(nc) as tc:
        with (
            tc.tile_pool(name="wp", bufs=1) as wp,
            tc.tile_pool(name="mixp", bufs=1) as mixp,
            tc.tile_pool(name="smallp", bufs=1) as smallp,
            tc.tile_pool(name="xp", bufs=6) as xp,
            tc.tile_pool(name="op", bufs=3) as op,
            tc.tile_pool(name="psp", bufs=6, space="PSUM") as psp,
            tc.tile_pool(name="psb", bufs=2, space="PSUM") as psb,
        ):
            # ---- small DMAs first (cheap SP issues, unblock bias/mixing) ----
            cbt = smallp.tile([128, G_PER_CORE * N_EXPERTS], F32R, tag="cb")
            nc.sync.dma_start(cbt[:], cb[:])
            cxt = smallp.tile([NSL, G_PER_CORE], F32R, tag="cx")
            nc.sync.dma_start(cxt[:], cx[:])
            ballt = smallp.tile([NSL, OUT_F], F32R, tag="ball")
            nc.sync.dma_start(ballt[:], ball[:])
            onest = smallp.tile([1, 128], F32R, tag="ones")
            nc.sync.dma_start(onest[:], ones[:])

            # ---- expert weights: 3 chunks of 3 slots, contiguous layout ----
            wtb = wp.tile([128, NSL * 2048], F32R, tag="wtb")
            for wch in range(3):
                lo, hi = wch * 3 * 2048, (wch + 1) * 3 * 2048
                nc.sync.dma_start(wtb[:, lo:hi], wt[:, lo:hi])

            # ---- x chunk loads (pool-limited prefetch pipeline) ----
            xts = []
            for ch in range(N_CH):
                xt = xp.tile([128, 2048], F32R, tag="xb")
                nc.sync.dma_start(xt[:], xs[:, ch * 2048:(ch + 1) * 2048])
                xts.append(xt)

            # ---- mixed biases: mb_g = cx[:, g].T @ ball  (K=9, M=1, N=512) ----
            mbt = []
            for g in range(G_PER_CORE):
                pbg = psb.tile([1, OUT_F], F32, tag="pb")
                nc.tensor.matmul(pbg[:], cxt[:, g:g + 1], ballt[:], start=True, stop=True)
                mb = smallp.tile([1, OUT_F], F32R, tag=f"mb{g}")
                nc.vector.tensor_copy(mb[:], pbg[:])
                mbt.append(mb)

            # ---- weight mixing on DVE: wm_g = sum_e c[g,e]*W_e + W_sh ----
            # slot s occupies wtb[:, s*2048:(s+1)*2048]; slot 0 is shared.
            wm = []
            for g in range(G_PER_CORE):
                wmg = mixp.tile([128, 2048], F32R, tag=f"wm{g}")
                wm.append(wmg)
            ws = lambda s: wtb[:, s * 2048:(s + 1) * 2048]
            for stage in range(3):
                for g in range(G_PER_CORE):
                    if stage == 0:
                        nc.vector.scalar_tensor_tensor(
                            wm[g][:], ws(1), cbt[:, g * 8:g * 8 + 1], ws(0),
                            AluOpType.mult, AluOpType.add,
                        )
                        ks = [1]
                    elif stage == 1:
                        ks = [2, 3, 4]
                    else:
                        ks = [5, 6, 7]
                    for k in ks:
                        nc.vector.scalar_tensor_tensor(
                            wm[g][:], ws(k + 1), cbt[:, g * 8 + k:g * 8 + k + 1],
                            wm[g][:], AluOpType.mult, AluOpType.add,
                        )

            # ---- main GEMM ----
            for ch in range(N_CH):
                g = ch // (N_CH // G_PER_CORE)
                xt = xts[ch]
                oc = op.tile([128, 4 * OUT_F], F32, tag="oc")
                for j in range(4):
                    ps = psp.tile([128, OUT_F], F32, tag="ps")
                    for kt in range(KT):
                        nc.tensor.matmul(
                            ps[:],
                            xt[:, (kt * 4 + j) * 128:(kt * 4 + j) * 128 + 128],
                            wm[g][:, kt * 512:(kt + 1) * 512],
                            start=(kt == 0),
                            stop=False,
                        )
                    nc.tensor.matmul(ps[:], onest[:], mbt[g][:], start=False, stop=True)
                    nc.scalar.copy(oc[:, j * OUT_F:(j + 1) * OUT_F], ps[:])
                nc.scalar.dma_start(
                    out[ch * 512:(ch + 1) * 512, :].rearrange("(p j) o -> p (j o)", p=128),
                    oc[:],
                )
    nc.finalize()
    return nc


def kernel(x, coefficients, weight_experts, bias_experts, weight_shared, bias_shared, sizes):
    x = np.asarray(x)
    coefficients = np.asarray(coefficients)
    weight_experts = np.asarray(weight_experts)
    bias_experts = np.asarray(bias_experts)
    weight_shared = np.asarray(weight_shared)
    bias_shared = np.asarray(bias_shared)

    if "nc" not in _CACHE:
        _CACHE["nc"] = _build()
    nc = _CACHE["nc"]

    # ---- host-side layout prep (transpose/stack/permute only) ----
    # weights: [kpart, slot, kt, o] with slot 0 = shared, 1..8 = experts
    warr = np.empty((NSL, IN_F, OUT_F), np.float32)
    warr[0] = weight_shared.T
    for e in range(N_EXPERTS):
        warr[1 + e] = weight_experts[e].T
    wt_np = np.ascontiguousarray(
        warr.reshape(NSL, KT, 128, OUT_F).transpose(2, 0, 1, 3)
    ).reshape(128, NSL * 2048)

    ball_np = np.empty((NSL, OUT_F), np.float32)
    ball_np[0] = bias_shared
    ball_np[1:] = bias_experts
    ones_np = np.ones((1, 128), np.float32)

    in_maps = []
    for c in range(N_CORES):
        gs = slice(c * G_PER_CORE, (c + 1) * G_PER_CORE)
        cg = coefficients[gs]  # [4, 8]
        cb_np = np.broadcast_to(
            cg.reshape(1, -1), (128, G_PER_CORE * N_EXPERTS)
        ).copy()
        cx_np = np.empty((NSL, G_PER_CORE), np.float32)
        cx_np[0] = 1.0
        cx_np[1:] = cg.T
        # x: [kpart, ch, kt, j, p] with chunk-local token t = 4p + j
        xc = x[c * TOK_PER_CORE:(c + 1) * TOK_PER_CORE]
        xs_np = np.ascontiguousarray(
            xc.reshape(N_CH, 128, 4, KT, 128).transpose(4, 0, 3, 2, 1)
        ).reshape(128, N_CH * 2048)
        in_maps.append(
            {
                "xs": xs_np,
                "wt": wt_np,
                "cb": cb_np,
                "cx": cx_np,
                "ball": ball_np,
                "ones": ones_np,
            }
        )

    res = run_bass_kernel_spmd(nc, in_maps, core_ids=list(range(N_CORES)))
    return np.concatenate([res.results[c]["out"] for c in range(N_CORES)], axis=0)


# revision 5
# speedup vs baseline: 1.7838x; 1.7838x over previous
"""MOLELinear (mixture-of-linear-experts) Trainium2 kernel, v4.

Math (per group g): out_g = x_g @ (sum_e c[g,e] W_e + W_sh).T + (sum_e c[g,e] b_e + b_sh)

Sharding: data-parallel over the 32 groups -> 4 groups (8192 tokens) per core,
expert weights replicated. Host does layout-only prep (transpose/stack/permute
plus a lossless bf16->f32 widen of the device output); all arithmetic runs on
device.

Design (from trace analysis of prior versions):
  - bf16 GEMM on PE (f32r matmuls measured ~3x slower than bf16 on HW).
  - DMA layouts are one contiguous 8KB/4KB segment per partition: x staged
    [kpart, ch, kt, j, p] with chunk-local token t = 4p + j, which also makes
    output rows per partition contiguous.
  - Weights staged [kpart, slot, kt, o] (slot 0 = shared, 1..8 = experts),
    loaded slot-by-slot, converted f32->bf16 on DVE, then mixed group-major
    (g0 completes first) as 8 bf16 STT FMAs of free-dim 2048 per group.
  - x chunks: f32 DMA + ScalarE cast to bf16.
  - Bias on PE: K=9 mix matmul per group, then a K=1 ones-row matmul adds the
    mixed bias into each PSUM accumulation group.
  - PSUM drained to bf16 on ScalarE; output staged bf16 and widened to f32 on
    the host (exact), halving output DMA bytes.
"""
import ml_dtypes
import numpy as np

import concourse.bacc as bacc
import concourse.mybir as mybir
from concourse.alu_op_type import AluOpType
from concourse.tile import TileContext
from concourse.bass_utils import run_bass_kernel_spmd

N_CORES = 8
IN_F = 512
OUT_F = 512
N_EXPERTS = 8
N_GROUPS = 32
TOK_PER_GROUP = 2048
G_PER_CORE = N_GROUPS // N_CORES          # 4
TOK_PER_CORE = G_PER_CORE * TOK_PER_GROUP  # 8192
KT = IN_F // 128                           # 4 k-tiles
N_CH = TOK_PER_CORE // 512                 # 16 chunks of 512 tokens
NSL = N_EXPERTS + 1                        # weight slots: [shared, e0..e7]
F32 = mybir.dt.float32
F32R = mybir.dt.float32r
BF16 = mybir.dt.bfloat16

_CACHE = {}


def _build():
    nc = bacc.Bacc(trn_type="TRN2")
    xs = nc.dram_tensor("xs", (128, N_CH * 2048), F32, kind="ExternalInput")
    wt = nc.dram_tensor("wt", (128, NSL * 2048), F32, kind="ExternalInput")
    cb = nc.dram_tensor("cb", (128, G_PER_CORE * N_EXPERTS), F32, kind="ExternalInput")
    cx = nc.dram_tensor("cx", (NSL, G_PER_CORE), F32R, kind="ExternalInput")
    ball = nc.dram_tensor("ball", (NSL, OUT_F), F32R, kind="ExternalInput")
    ones = nc.dram_tensor("ones", (1, 128), BF16, kind="ExternalInput")
    out = nc.dram_tensor("out", (TOK_PER_CORE, OUT_F), BF16, kind="ExternalOutput")

    with TileContext(nc) as tc:
        with (
            tc.tile_pool(name="wbp", bufs=1) as wbp,
            tc.tile_pool(name="wsp", bufs=3) as wsp,
            tc.tile_pool(name="mixp", bufs=1) as mixp,
            tc.tile_pool(name="smallp", bufs=1) as smallp,
            tc.tile_pool(name="xp", bufs=6) as xp,
            tc.tile_pool(name="xbp", bufs=4) as xbp,
            tc.tile_pool(name="op", bufs=3) as op,
            tc.tile_pool(name="psp", bufs=3, space="PSUM") as psp,
            tc.tile_pool(name="psb", bufs=2, space="PSUM") as psb,
        ):
            # ---- small DMAs first (cheap SP issues, unblock bias/mixing) ----
            cbt = smallp.tile([128, G_PER_CORE * N_EXPERTS], F32, tag="cb")
            nc.sync.dma_start(cbt[:], cb[:])
            cxt = smallp.tile([NSL, G_PER_CORE], F32R, tag="cx")
            nc.sync.dma_start(cxt[:], cx[:])
            ballt = smallp.tile([NSL, OUT_F], F32R, tag="ball")
            nc.sync.dma_start(ballt[:], ball[:])
            onest = smallp.tile([1, 128], BF16, tag="ones")
            nc.sync.dma_start(onest[:], ones[:])

            # ---- expert weights: 9 slot DMAs (f32 staging, bufs=3) ----
            wst = []
            for s in range(NSL):
                wsf = wsp.tile([128, 2048], F32, tag="wsf")
                nc.sync.dma_start(wsf[:], wt[:, s * 2048:(s + 1) * 2048])
                wst.append(wsf)

            # ---- x chunk loads (f32, pool-limited prefetch pipeline) ----
            xts = []
            for ch in range(N_CH):
                xt = xp.tile([128, 2048], F32, tag="xf")
                nc.sync.dma_start(xt[:], xs[:, ch * 2048:(ch + 1) * 2048])
                xts.append(xt)

            # ---- mixed biases: mb_g = cx[:, g].T @ ball  (K=9, M=1, N=512) ----
            mbt = []
            for g in range(G_PER_CORE):
                pbg = psb.tile([1, OUT_F], F32, tag="pb")
                nc.tensor.matmul(pbg[:], cxt[:, g:g + 1], ballt[:], start=True, stop=True)
                mb = smallp.tile([1, OUT_F], BF16, tag=f"mb{g}")
                nc.vector.tensor_copy(mb[:], pbg[:])
                mbt.append(mb)

            # ---- coefficients to bf16 for 2x-mode STT ----
            cbb = smallp.tile([128, G_PER_CORE * N_EXPERTS], BF16, tag="cbb")
            nc.vector.tensor_copy(cbb[:], cbt[:])

            # ---- weight slot converts f32->bf16 on DVE, interleaved with
            # ---- g0's mixing chain so g0 completes as slot 8 lands ----
            wtb = wbp.tile([128, NSL * 2048], BF16, tag="wtb")
            ws = lambda s: wtb[:, s * 2048:(s + 1) * 2048]
            wm = []
            for g in range(G_PER_CORE):
                wmg = mixp.tile([128, 2048], BF16, tag=f"wm{g}")
                wm.append(wmg)

            def mix_op(g, k):
                # acc op k: wm_g = c[g,k]*W_k + (W_sh if k==0 else wm_g)
                nc.vector.scalar_tensor_tensor(
                    wm[g][:], ws(k + 1), cbb[:, g * 8 + k:g * 8 + k + 1],
                    wm[g][:] if k else ws(0), AluOpType.mult, AluOpType.add,
                )

            nc.vector.tensor_copy(ws(0), wst[0][:])
            nc.vector.tensor_copy(ws(1), wst[1][:])
            mix_op(0, 0)
            for k in range(1, N_EXPERTS):
                nc.vector.tensor_copy(ws(k + 1), wst[k + 1][:])
                mix_op(0, k)
            for g in range(1, G_PER_CORE):
                for k in range(N_EXPERTS):
                    mix_op(g, k)

            # ---- main GEMM; x-cast for chunk ch+2 is emitted ahead of chunk
            # ---- ch's drains so ScalarE's FIFO never stalls the PE ----
            xbs = []

            def cast_x(ch):
                xb = xbp.tile([128, 2048], BF16, tag="xb")
                nc.scalar.copy(xb[:], xts[ch][:])
                xbs.append(xb)

            cast_x(0)
            cast_x(1)
            for ch in range(N_CH):
                g = ch // (N_CH // G_PER_CORE)
                xb = xbs[ch]
                oc = op.tile([128, 4 * OUT_F], BF16, tag="oc")
                ps2 = []
                for jp in range(2):
                    ps = psp.tile([128, 2 * OUT_F], F32, tag="ps")
                    ps2.append(ps)
                    for j2 in range(2):
                        j = jp * 2 + j2
                        half = ps[:, j2 * OUT_F:(j2 + 1) * OUT_F]
                        for kt in range(KT):
                            nc.tensor.matmul(
                                half,
                                xb[:, (kt * 4 + j) * 128:(kt * 4 + j) * 128 + 128],
                                wm[g][:, kt * 512:(kt + 1) * 512],
                                start=(kt == 0),
                                stop=False,
                            )
                        nc.tensor.matmul(half, onest[:], mbt[g][:], start=False, stop=True)
                if ch + 2 < N_CH:
                    cast_x(ch + 2)
                for jp in range(2):
                    nc.scalar.copy(oc[:, jp * 1024:(jp + 1) * 1024], ps2[jp][:])
                nc.scalar.dma_start(
                    out[ch * 512:(ch + 1) * 512, :].rearrange("(p j) o -> p (j o)", p=128),
                    oc[:],
                )
    nc.finalize()
    return nc


def kernel(x, coefficients, weight_experts, bias_experts, weight_shared, bias_shared, sizes):
    x = np.asarray(x)
    coefficients = np.asarray(coefficients)
    weight_experts = np.asarray(weight_experts)
    bias_experts = np.asarray(bias_experts)
    weight_shared = np.asarray(weight_shared)
    bias_shared = np.asarray(bias_shared)

    if "nc" not in _CACHE:
        _CACHE["nc"] = _build()
    nc = _CACHE["nc"]

    # ---- host-side layout prep (transpose/stack/permute only) ----
    # weights: [kpart, slot, kt, o] with slot 0 = shared, 1..8 = experts
    warr = np.empty((NSL, IN_F, OUT_F), np.float32)
    warr[0] = weight_shared.T
    for e in range(N_EXPERTS):
        warr[1 + e] = weight_experts[e].T
    wt_np = np.ascontiguousarray(
        warr.reshape(NSL, KT, 128, OUT_F).transpose(2, 0, 1, 3)
    ).reshape(128, NSL * 2048)

    ball_np = np.empty((NSL, OUT_F), np.float32)
    ball_np[0] = bias_shared
    ball_np[1:] = bias_experts
    ones_np = np.ones((1, 128), ml_dtypes.bfloat16)

    in_maps = []
    for c in range(N_CORES):
        gs = slice(c * G_PER_CORE, (c + 1) * G_PER_CORE)
        cg = coefficients[gs]  # [4, 8]
        cb_np = np.broadcast_to(
            cg.reshape(1, -1), (128, G_PER_CORE * N_EXPERTS)
        ).copy()
        cx_np = np.empty((NSL, G_PER_CORE), np.float32)
        cx_np[0] = 1.0
        cx_np[1:] = cg.T
        # x: [kpart, ch, kt, j, p] with chunk-local token t = 4p + j
        xc = x[c * TOK_PER_CORE:(c + 1) * TOK_PER_CORE]
        xs_np = np.ascontiguousarray(
            xc.reshape(N_CH, 128, 4, KT, 128).transpose(4, 0, 3, 2, 1)
        ).reshape(128, N_CH * 2048)
        in_maps.append(
            {
                "xs": xs_np,
                "wt": wt_np,
                "cb": cb_np,
                "cx": cx_np,
                "ball": ball_np,
                "ones": ones_np,
            }
        )

    res = run_bass_kernel_spmd(nc, in_maps, core_ids=list(range(N_CORES)))
    return np.concatenate(
        [res.results[c]["out"] for c in range(N_CORES)], axis=0
    ).astype(np.float32)
